# revision 30
# baseline (speedup 1.0000x reference)
"""Trainium2 Bass kernel for ActorMoE (8 experts, dims 512->1024->512->256->64).

Strategy: data-parallel across 8 NeuronCores (2048 rows each), weights
replicated. On-device compute is feature-major (features on partitions,
batch on the free dim) so the stacked expert weights W_l[e] (shape
[in, out]) are directly the matmul lhsT and no transposes are needed.

ELU trick: h' = elu(z)+1 = min(exp(z+b), max(z+(b+1), 1)), computed as
  e = Exp(z + b)                       (ScalarE, bias fused)
  h' = min(e, max(z + (b+1), 1))       (one custom DVE op: ELU_P1_MOE)
The +1 shift is corrected by subtracting colsum(W_next) from the next
layer's bias on the host, so the math is exact.

Matmuls are emitted with same-weight run length 4 (both 2-bank PSUM groups
of an m-tile accumulate together). Expert layers are software-pipelined
(tick t: load(t+1), L0(t), L2(t-1), L1(t), L3pair at even t) so each
layer-boundary ELU drain is covered by other matmul work; the gate fills
the first boundary.

L3 (256->64) is emitted per expert PAIR with column-tiled matmuls: expert
2p writes PSUM partitions 0-63, expert 2p+1 partitions 64-127
(tile_position col groups) so the two streams execute concurrently on the
PE array — M=64 alone would leave half the array idle. Only the bank's
first matmul uses start=True (bank-wide has_written clear); the second
expert's first write relies on unset has_written bits = overwrite.
The weighted accumulation over experts lands in acc[128, BSH] (even
experts in partitions 0-63, odd in 64-127); the host adds the two halves.

Per-expert weights/biases are packed into 3 DMA transfers (W0|W1 blob,
W2|W3 blob, bias blob) — DMA enqueue instructions cost ~600ns of
sequencer time each, so fewer/bigger transfers shorten the kernel's
startup ramp.

Softmax gate: logits are small (|logit| < ~2) so exp without max-shift is
safe. Per-expert gate rows are replicated across partitions via broadcast
DMA (bounced through DRAM, since partition-broadcast needs a DRAM source).
"""

import os
import sys

sys.path.insert(0, "/opt/trn_rl_repo")

import numpy as np
import ml_dtypes

BF = ml_dtypes.bfloat16

B, OBS, ACT, E = 16384, 512, 64, 8
DIMS = [512, 1024, 512, 256, 64]
GH = 256
NCORES = 8
BSH = B // NCORES  # 2048
P = 128
FD = 512  # matmul free dim (one PSUM bank of f32)
NT = BSH // FD  # 4 n-tiles per core
NB = int(os.environ.get("NB_ENV", "2"))  # PSUM banks per group
NG = NT // NB  # groups per m-tile

KTS = [DIMS[l] // P for l in range(4)]  # [4, 8, 4, 2]
MTS = [DIMS[l + 1] // P for l in range(3)]  # [8, 4, 2]
# bias blob column layout: B0, B0p1, B1, B1p1, B2, B2p1
_BOFF = [0, 8, 16, 20, 24, 28]
WA_W = KTS[0] * DIMS[1] + KTS[1] * DIMS[2]  # 8192
WB_W = KTS[2] * DIMS[3] + KTS[3] * DIMS[4]  # 1152

_cache = {}


def _get_elu_op():
    """Custom DVE op: out = min(in1, max(in0 + s0, 1)).
    With in0 = z (PSUM), s0 = b+1 per-partition, in1 = exp(z+b) from ACT,
    this computes elu(z+b)+1 in a single DVE pass."""
    if "elu_op" in _cache:
        return _cache["elu_op"]
    from concourse.dve_ops import DveOp, OPS
    from concourse.dve_spec import Spec, Src0, Src1, C0, One, maxx, minn, lower
    from concourse.dve_uop import DveOpSpec

    spec = Spec(
        body=minn(Src1, maxx(Src0 + C0, One)),
        reference=lambda in0, in1, s0: np.minimum(
            in1, np.maximum(in0 + s0, 1.0)
        ),
    )
    shas = {}
    for ver in ("v3", "v4"):
        s = DveOpSpec(name="ELU_P1_MOE", opcode=0, uops=lower(spec, ver=ver), rd1_en=True)
        shas[ver] = s.sha(ver)
    op = DveOp("ELU_P1_MOE", spec, subdim=False, uops_sha=shas)
    OPS.append(op)
    # import-time lookup tables don't see post-import appends — patch them
    import concourse.dve_ops as dve_ops_mod

    dve_ops_mod.CUSTOM_DVE_SPECS[op.name] = op.spec
    dve_ops_mod._SUB_OPCODE_FOR_NAME[op.name] = (
        dve_ops_mod._CUSTOM_DVE_ROW_BASE + len(OPS) - 1
    )
    _cache["elu_op"] = op
    return op


def _build(reps=1, nb=NB, elu=None, dq=0, eb=0):
    """Build the Bass graph. reps>1 wraps the whole body in a For_i loop
    (the body is idempotent) — used only for timing via wall-time slope."""
    import concourse.bass as bass  # noqa: F401
    from concourse import bacc, mybir
    import concourse.tile as tile

    _elu_override = elu

    NB = nb
    NG = NT // NB
    PS_BUFS = 8 // NB
    E_BUFS = (PS_BUFS + 2) if not eb else eb

    f32 = mybir.dt.float32
    bf16 = mybir.dt.bfloat16
    AF = mybir.ActivationFunctionType
    Alu = mybir.AluOpType

    nc = bacc.Bacc(None, target_bir_lowering=False)

    xTd = nc.dram_tensor("xT", [OBS, BSH], bf16, kind="ExternalInput")
    WAd = nc.dram_tensor("WA", [E, P, WA_W], bf16, kind="ExternalInput")
    WBd = nc.dram_tensor("WB", [E, P, WB_W], bf16, kind="ExternalInput")
    BIASd = nc.dram_tensor("BIAS", [E, P, 32], f32, kind="ExternalInput")
    gW0d = nc.dram_tensor("gW0", [OBS, GH], bf16, kind="ExternalInput")
    gW1d = nc.dram_tensor("gW1", [GH, E], bf16, kind="ExternalInput")
    # packed small constants: gb0 | gb0+1 | b3 pairs | gb1
    GSMd = nc.dram_tensor("GSM", [P, 9], f32, kind="ExternalInput")
    # out = pairs 0-2 accumulator, out2 = last pair's weighted term
    # (host sums all four 64-partition halves)
    outd = nc.dram_tensor("out", [P, BSH], f32, kind="ExternalOutput")
    out2d = nc.dram_tensor("out2", [P, BSH], f32, kind="ExternalOutput")

    with tile.TileContext(nc) as tc:
        with (
            tc.tile_pool(name="const", bufs=1) as cpool,
            tc.tile_pool(name="wapool", bufs=2) as wapool,
            tc.tile_pool(name="wbpool", bufs=4) as wbpool,
            tc.tile_pool(name="bpool", bufs=4) as bpool,
            tc.tile_pool(name="rwpool", bufs=2) as rwpool,
            tc.tile_pool(name="hpool", bufs=1) as hpool,
            tc.tile_pool(name="h3pool", bufs=2) as h3pool,
            tc.tile_pool(name="epool", bufs=E_BUFS) as epool,
            tc.tile_pool(name="rpool", bufs=4) as rpool,
            tc.tile_pool(name="tpool", bufs=2) as tpool,
            tc.tile_pool(name="psum", bufs=PS_BUFS, space="PSUM") as pspool,
            tc.tile_pool(name="dram", bufs=1, space="DRAM") as dpool,
        ):

            def body():
                # ---- load x and gate params ----
                # xt slices split across BOTH HW rings (scalar: k=0,1,3;
                # sync: k=2 ahead of the WA blob) so the gate's k-outer burn
                # rate matches slice arrivals and expert 0's weights land
                # right as the gate finishes — zero-gap handoff.
                gw0 = cpool.tile([P, OBS // P, GH], bf16, tag="gw0", name="gw0")
                nc.sync.dma_start(gw0[:], gW0d[:].rearrange("(ko p) o -> p ko o", p=P))
                xt = cpool.tile([P, OBS // P, BSH], bf16, tag="xt", name="xt")
                xt_src = xTd[:].rearrange("(ko p) n -> p ko n", p=P)
                for ko in range(OBS // P):
                    q = nc.sync if ko == 2 else nc.scalar
                    q.dma_start(xt[:, ko : ko + 1, :], xt_src[:, ko : ko + 1, :])
                gsm = cpool.tile([P, 9], f32, tag="gsm", name="gsm")
                nc.scalar.dma_start(gsm[:], GSMd[:])
                gw1 = cpool.tile([P, GH // P, E], bf16, tag="gw1", name="gw1")
                gb0t = gsm[:, 0:2]
                gb0p1t = gsm[:, 2:4]
                b3pt = gsm[:, 4:8]
                gb1t = gsm[0:E, 8:9]

                import os

                elu_mode = _elu_override or os.environ.get("ELU_MODE", "cdve")
                elu_op = _get_elu_op() if elu_mode == "cdve" else None

                def elu_wide(ps_flat, bias_ap, biasp1_ap, out_ap, mp=P):
                    # ps_flat: [mp, NB*FD] PSUM view.
                    if elu_mode == "cdve":
                        # one wide ACT + one custom DVE from PSUM:
                        # h' = min(exp(z+b), max(z+(b+1), 1)) = elu(z+b)+1
                        et = epool.tile([P, NB * FD], bf16, tag="e", name="e")[:mp]
                        nc.scalar.activation(et, ps_flat, AF.Exp, bias=bias_ap)
                        nc.vector._custom_dve(
                            elu_op, out=out_ap, in0=ps_flat, in1=et, s0=biasp1_ap
                        )
                        return
                    # elu(z+b)+1 = min(exp(z+b), 1) + relu(z+b): two ACT passes
                    # over PSUM (released early) + one all-bf16 DVE merge
                    et = epool.tile([P, NB * FD], bf16, tag="e", name="e")[:mp]
                    nc.scalar.activation(et, ps_flat, AF.Exp, bias=bias_ap)
                    rt = rpool.tile([P, NB * FD], bf16, tag="r", name="r")[:mp]
                    nc.scalar.activation(rt, ps_flat, AF.Relu, bias=bias_ap)
                    nc.vector.scalar_tensor_tensor(
                        out_ap, et, 1.0, rt, Alu.min, Alu.add
                    )

                def psum_mm_groups(win_col, rhs_tile, KT, mp=P):
                    """All NG groups of one m-tile accumulated together so each
                    weight load serves NT consecutive matmuls (same-weight run
                    length 4). Returns one flat [mp, NB*FD] view per group."""
                    psts = [
                        pspool.tile([P, NB, FD], f32, tag="ps", name="ps")
                        for _ in range(NG)
                    ]
                    for k in range(KT):
                        lhs = win_col(k)
                        for g in range(NG):
                            for n in range(NB):
                                ng = g * NB + n
                                nc.tensor.matmul(
                                    psts[g][:mp, n, :],
                                    lhs,
                                    rhs_tile[:, k, ng * FD : (ng + 1) * FD],
                                    start=(k == 0),
                                    stop=(k == KT - 1),
                                )
                    return [pst[:mp].rearrange("p a b -> p (a b)") for pst in psts]

                def layer(win, bt, btp1, KT, MT, rhs_tile, out_tile):
                    """z = win.T @ rhs + b; out = elu(z)+1 (bf16)."""
                    for m in range(MT):
                        flats = psum_mm_groups(
                            lambda k, m=m: win[:, k, m * P : (m + 1) * P],
                            rhs_tile,
                            KT,
                        )
                        for g in range(NG):
                            elu_wide(
                                flats[g],
                                bt[:, m : m + 1],
                                btp1[:, m : m + 1],
                                out_tile[:, m, g * NB * FD : (g + 1) * NB * FD],
                            )

                def emit_gate_l1():
                    # gate layer 1 (512 -> 256, elu'), k-outer across BOTH
                    # m-tiles (4 PSUM groups): each xt k-slice is consumed in
                    # one 8-MM burst so the matmuls keep pace with the
                    # k-sliced xt DMA arrivals at kernel start.
                    gp = cpool.tile([P, GH // P, BSH], bf16, tag="gp", name="gp")
                    MT = GH // P
                    KT = OBS // P
                    psts = [
                        pspool.tile([P, NB, FD], f32, tag="ps", name="ps")
                        for _ in range(MT * NG)
                    ]
                    for k in range(KT):
                        for m in range(MT):
                            lhs = gw0[:, k, m * P : (m + 1) * P]
                            for g in range(NG):
                                for n in range(NB):
                                    ng = g * NB + n
                                    nc.tensor.matmul(
                                        psts[m * NG + g][:, n, :],
                                        lhs,
                                        xt[:, k, ng * FD : (ng + 1) * FD],
                                        start=(k == 0),
                                        stop=(k == KT - 1),
                                    )
                    for m in range(MT):
                        for g in range(NG):
                            elu_wide(
                                psts[m * NG + g].rearrange("p a b -> p (a b)"),
                                gb0t[:, m : m + 1],
                                gb0p1t[:, m : m + 1],
                                gp[:, m, g * NB * FD : (g + 1) * NB * FD],
                            )
                    return gp

                def emit_gate_rest(gp):
                    # gate layer 2 (256 -> 8) + exp
                    expT = cpool.tile([E, BSH], f32, tag="expT", name="expT")
                    gflats = psum_mm_groups(lambda k: gw1[:, k, :], gp, GH // P, mp=E)
                    for g in range(NG):
                        nc.scalar.activation(
                            expT[:, g * NB * FD : (g + 1) * NB * FD],
                            gflats[g],
                            AF.Exp,
                            bias=gb1t[:, 0:1],
                        )
                    # softmax denom: sum over 8 experts via ones-matmul
                    ones = cpool.tile([E, 1], f32, tag="ones", name="ones")
                    nc.vector.memset(ones[:], 1.0)
                    invs = cpool.tile([1, BSH], f32, tag="invs", name="invs")
                    sflats = psum_mm_groups(lambda k: ones[:], expT[:, None, :], 1, mp=1)
                    for g in range(NG):
                        nc.vector.reciprocal(
                            invs[:, g * NB * FD : (g + 1) * NB * FD], sflats[g]
                        )
                    # wT[e, s] = exp(logit_e)/sum (partition-broadcast DMA
                    # needs a DRAM source, so bounce via DRAM)
                    inv_d = dpool.tile([1, BSH], f32, name="inv_d")
                    nc.scalar.dma_start(inv_d[:], invs[:])
                    rep8 = cpool.tile([E, BSH], f32, tag="rep8", name="rep8")
                    nc.scalar.dma_start(
                        rep8[:], inv_d[0:1, :].to_broadcast((E, BSH))
                    )
                    wT = cpool.tile([E, BSH], bf16, tag="wT", name="wT")
                    nc.vector.tensor_tensor(wT[:], expT[:], rep8[:], Alu.mult)
                    wt_d = dpool.tile([E, BSH], bf16, name="wt_d")
                    nc.scalar.dma_start(wt_d[:], wT[:])
                    return wt_d

                def load_expert(e):
                    st = {}
                    wa = wapool.tile([P, WA_W], bf16, tag="wa", name="wa")
                    # W0 half first as its own transfer: L0 only needs W0,
                    # so expert 0's first layer starts ~3us sooner at kernel
                    # start / rep boundary (W1 follows before L1 needs it).
                    w0w = KTS[0] * DIMS[1]
                    nc.sync.dma_start(wa[:, 0:w0w], WAd[e][:, 0:w0w])
                    q2 = nc.scalar if dq else nc.sync
                    q2.dma_start(wa[:, w0w:WA_W], WAd[e][:, w0w:WA_W])
                    wb = wbpool.tile([P, WB_W], bf16, tag="wb", name="wb")
                    nc.sync.dma_start(wb[:], WBd[e])
                    bt = bpool.tile([P, 32], f32, tag="bias", name="bias")
                    nc.scalar.dma_start(bt[:], BIASd[e])
                    off = KTS[0] * DIMS[1]
                    st["w0"] = wa[:, 0:off].rearrange("p (k o) -> p k o", k=KTS[0])
                    st["w1"] = wa[:, off:WA_W].rearrange("p (k o) -> p k o", k=KTS[1])
                    off = KTS[2] * DIMS[3]
                    st["w2"] = wb[:, 0:off].rearrange("p (k o) -> p k o", k=KTS[2])
                    st["w3"] = wb[:, off:WB_W].rearrange("p (k o) -> p k o", k=KTS[3])
                    st["bts"] = [
                        (
                            bt[:, _BOFF[2 * l] : _BOFF[2 * l] + MTS[l]],
                            bt[:, _BOFF[2 * l + 1] : _BOFF[2 * l + 1] + MTS[l]],
                        )
                        for l in range(3)
                    ]
                    return st

                def emit_L0(st):
                    st["h1"] = hpool.tile(
                        [P, DIMS[1] // P, BSH], bf16, tag="h1", name="h1"
                    )
                    layer(
                        st["w0"], st["bts"][0][0], st["bts"][0][1],
                        KTS[0], DIMS[1] // P, xt, st["h1"],
                    )

                def emit_L1(st):
                    st["h2"] = hpool.tile(
                        [P, DIMS[2] // P, BSH], bf16, tag="h2", name="h2"
                    )
                    layer(
                        st["w1"], st["bts"][1][0], st["bts"][1][1],
                        KTS[1], DIMS[2] // P, st["h1"], st["h2"],
                    )

                def emit_L2(st):
                    st["h3"] = h3pool.tile(
                        [P, DIMS[3] // P, BSH], bf16, tag="h3", name="h3"
                    )
                    layer(
                        st["w2"], st["bts"][2][0], st["bts"][2][1],
                        KTS[2], DIMS[3] // P, st["h2"], st["h3"],
                    )

                def emit_rw_pair(pair, wt_d):
                    """Gate rows for experts (2p, 2p+1) replicated over the
                    two 64-partition halves; prefetched a tick early."""
                    eA, eB = 2 * pair, 2 * pair + 1
                    rw = rwpool.tile([P, BSH], bf16, tag="rw", name="rw")
                    nc.scalar.dma_start(
                        rw[0:ACT, :], wt_d[eA : eA + 1, :].to_broadcast((ACT, BSH))
                    )
                    nc.scalar.dma_start(
                        rw[ACT:P, :], wt_d[eB : eB + 1, :].to_broadcast((ACT, BSH))
                    )
                    return rw

                def emit_L3_pair(stA, stB, pair, acc, rw):
                    """L3 (256 -> 64) for experts (2p, 2p+1), column-tiled:
                    expert A -> PSUM partitions 0-63, B -> 64-127 so the two
                    matmul streams execute concurrently on the PE array.
                    acc[0:64] accumulates even experts, acc[64:128] odd."""
                    psts = [
                        pspool.tile([P, NB, FD], f32, tag="ps", name="ps")
                        for _ in range(NG)
                    ]
                    KT = KTS[3]
                    # Per bank: A's full k-accumulation first, then B's with
                    # its own start=True — the bank-wide has_written clear
                    # doesn't erase A's finished data and nothing rewrites
                    # A's partitions after. Adjacent banks interleave A/B so
                    # the two col-groups still overlap on the array.
                    for g in range(NG):
                        for n in range(NB):
                            ng = g * NB + n
                            rs = slice(ng * FD, (ng + 1) * FD)
                            for st_, base in ((stA, 0), (stB, ACT)):
                                for k in range(KT):
                                    nc.tensor.matmul(
                                        psts[g][base : base + ACT, n, :],
                                        st_["w3"][:, k, :],
                                        st_["h3"][:, k, rs],
                                        start=(k == 0),
                                        stop=(k == KT - 1),
                                        skip_group_check=True,
                                    )
                    for g in range(NG):
                        gs = slice(g * NB * FD, (g + 1) * NB * FD)
                        ps_flat = psts[g].rearrange("p a b -> p (a b)")
                        if pair == 0:
                            nc.vector.scalar_tensor_tensor(
                                acc[:, gs], ps_flat, b3pt[:, pair : pair + 1],
                                rw[:, gs], Alu.add, Alu.mult,
                            )
                        else:
                            tt = tpool.tile([P, NB * FD], f32, tag="t", name="t")
                            nc.vector.scalar_tensor_tensor(
                                tt, ps_flat, b3pt[:, pair : pair + 1],
                                rw[:, gs], Alu.add, Alu.mult,
                            )
                            if pair == E // 2 - 1:
                                # last pair: skip the accumulate — its term
                                # streams out on its own tensor (host sums),
                                # shortening the end-of-kernel DVE drain.
                                nc.scalar.dma_start(out2d[:, gs], tt[:])
                            else:
                                nc.vector.tensor_add(acc[:, gs], acc[:, gs], tt)

                acc = cpool.tile([P, BSH], f32, tag="acc", name="acc")

                # software pipeline over ticks t=0..E; L3 per expert pair at
                # even t covers the (t-2, t-1) experts whose h3 are both ready.
                state = {0: load_expert(0)}
                # gw1 is small and first needed at gate_rest — load it after
                # expert 0's blobs so they don't wait behind it on the ring.
                nc.sync.dma_start(gw1[:], gW1d[:].rearrange("(ko p) o -> p ko o", p=P))
                wt_d = None
                rw = None
                for t in range(E + 1):
                    if t == 0:
                        gp = emit_gate_l1()
                    if t < E:
                        if t + 1 < E:
                            state[t + 1] = load_expert(t + 1)
                        emit_L0(state[t])
                    if t == 0:
                        wt_d = emit_gate_rest(gp)
                    if t % 2 == 1:
                        # prefetch next pair's gate rows a tick early
                        rw = emit_rw_pair((t - 1) // 2, wt_d)
                    if t >= 1:
                        emit_L2(state[t - 1])
                    if t < E:
                        emit_L1(state[t])
                    if t == E:
                        # acc is final after pair 2 (tick 6) — stream it out
                        # under the last tick's compute.
                        nc.scalar.dma_start(outd[:], acc[:])
                    if t >= 2 and t % 2 == 0:
                        emit_L3_pair(
                            state[t - 2], state[t - 1], (t - 2) // 2, acc, rw
                        )
                        del state[t - 2], state[t - 1]

            if reps == 1:
                body()
            else:
                with tc.For_i(0, reps, 1):
                    body()

    nc.compile()
    return nc


def _prep_inputs(inputs):
    """Host-side: shard/transposes/casts + bias folding. Returns in_maps."""
    x = np.asarray(inputs["x"], np.float32)
    Ws = [np.asarray(inputs[f"W{l}"], np.float32) for l in range(4)]
    bs = [np.asarray(inputs[f"b{l}"], np.float32) for l in range(4)]
    gW0 = np.asarray(inputs["gW0"], np.float32)
    gb0 = np.asarray(inputs["gb0"], np.float32)
    gW1 = np.asarray(inputs["gW1"], np.float32)
    gb1 = np.asarray(inputs["gb1"], np.float32)

    shared = {}

    def kmajor(w, l):
        # [in, out] -> [P, KT, out] -> [P, KT*out]
        return (
            w.reshape(KTS[l], P, DIMS[l + 1]).transpose(1, 0, 2).reshape(P, -1)
        )

    wa = np.concatenate(
        [
            np.stack([kmajor(Ws[0][e], 0) for e in range(E)]),
            np.stack([kmajor(Ws[1][e], 1) for e in range(E)]),
        ],
        axis=2,
    )
    shared["WA"] = np.ascontiguousarray(wa.astype(BF))
    wb = np.concatenate(
        [
            np.stack([kmajor(Ws[2][e], 2) for e in range(E)]),
            np.stack([kmajor(Ws[3][e], 3) for e in range(E)]),
        ],
        axis=2,
    )
    shared["WB"] = np.ascontiguousarray(wb.astype(BF))

    # effective biases: layer l>0 consumes h' = elu+1, so subtract colsum(W_l)
    beff = [bs[0]] + [bs[l] - Ws[l].sum(axis=1) for l in range(1, 4)]
    bias = np.zeros((E, P, 32), np.float32)
    for l in range(3):
        pk = beff[l].reshape(E, MTS[l], P).transpose(0, 2, 1)
        bias[:, :, _BOFF[2 * l] : _BOFF[2 * l] + MTS[l]] = pk
        bias[:, :, _BOFF[2 * l + 1] : _BOFF[2 * l + 1] + MTS[l]] = pk + 1.0
    shared["BIAS"] = np.ascontiguousarray(bias)
    shared["gW0"] = np.ascontiguousarray(gW0.astype(BF))
    shared["gW1"] = np.ascontiguousarray(gW1.astype(BF))
    # packed small constants: gb0 | gb0+1 | b3 pairs | gb1
    gsm = np.zeros((P, 9), np.float32)
    gpk = gb0.reshape(GH // P, P).T
    gsm[:, 0:2] = gpk
    gsm[:, 2:4] = gpk + 1.0
    for p_ in range(E // 2):
        gsm[0:ACT, 4 + p_] = beff[3][2 * p_]
        gsm[ACT:P, 4 + p_] = beff[3][2 * p_ + 1]
    gsm[0:E, 8] = gb1 - gW1.sum(axis=0)
    shared["GSM"] = np.ascontiguousarray(gsm)

    in_maps = []
    for c in range(NCORES):
        m = dict(shared)
        m["xT"] = np.ascontiguousarray(x[c * BSH : (c + 1) * BSH].T.astype(BF))
        in_maps.append(m)
    return in_maps


def kernel(**inputs):
    from concourse.bass_utils import run_bass_kernel_spmd

    if "nc" not in _cache:
        _cache["nc"] = _build()
    nc = _cache["nc"]
    in_maps = _prep_inputs(inputs)
    res = run_bass_kernel_spmd(nc, in_maps, core_ids=list(range(NCORES)))
    full = np.empty((B, ACT), np.float32)
    for c in range(NCORES):
        o = np.asarray(res.results[c]["out"])
        o2 = np.asarray(res.results[c]["out2"])
        full[c * BSH : (c + 1) * BSH] = (
            o[0:ACT] + o[ACT:P] + o2[0:ACT] + o2[ACT:P]
        ).T
    return full



# revision 32
# speedup vs baseline: 1.0395x; 1.0395x over previous
"""Trainium2 Bass kernel for ActorMoE (8 experts, dims 512->1024->512->256->64).

Strategy: data-parallel across 8 NeuronCores (2048 rows each), weights
replicated. On-device compute is feature-major (features on partitions,
batch on the free dim) so the stacked expert weights W_l[e] (shape
[in, out]) are directly the matmul lhsT and no transposes are needed.

ELU trick: h' = elu(z)+1 = min(exp(z+b), max(z+(b+1), 1)), computed as
  e = Exp(z + b)                       (ScalarE, bias fused)
  h' = min(e, max(z + (b+1), 1))       (one custom DVE op: ELU_P1_MOE)
The +1 shift is corrected by subtracting colsum(W_next) from the next
layer's bias on the host, so the math is exact.

Matmuls are emitted with same-weight run length 4 (both 2-bank PSUM groups
of an m-tile accumulate together). Expert layers are software-pipelined
(tick t: load(t+1), L0(t), L2(t-1), L1(t), L3pair at even t) so each
layer-boundary ELU drain is covered by other matmul work; the gate fills
the first boundary.

L3 (256->64) is emitted per expert PAIR with column-tiled matmuls: expert
2p writes PSUM partitions 0-63, expert 2p+1 partitions 64-127
(tile_position col groups) so the two streams execute concurrently on the
PE array — M=64 alone would leave half the array idle. Only the bank's
first matmul uses start=True (bank-wide has_written clear); the second
expert's first write relies on unset has_written bits = overwrite.
The weighted accumulation over experts lands in acc[128, BSH] (even
experts in partitions 0-63, odd in 64-127); the host adds the two halves.

Per-expert weights/biases are packed into 3 DMA transfers (W0|W1 blob,
W2|W3 blob, bias blob) — DMA enqueue instructions cost ~600ns of
sequencer time each, so fewer/bigger transfers shorten the kernel's
startup ramp.

Softmax gate: logits are small (|logit| < ~2) so exp without max-shift is
safe. Per-expert gate rows are replicated across partitions via broadcast
DMA (bounced through DRAM, since partition-broadcast needs a DRAM source).
"""

import os
import sys

sys.path.insert(0, "/opt/trn_rl_repo")

import numpy as np
import ml_dtypes

BF = ml_dtypes.bfloat16

B, OBS, ACT, E = 16384, 512, 64, 8
DIMS = [512, 1024, 512, 256, 64]
GH = 256
NCORES = 8
BSH = B // NCORES  # 2048
P = 128
FD = 512  # matmul free dim (one PSUM bank of f32)
NT = BSH // FD  # 4 n-tiles per core
NB = int(os.environ.get("NB_ENV", "2"))  # PSUM banks per group
NG = NT // NB  # groups per m-tile

KTS = [DIMS[l] // P for l in range(4)]  # [4, 8, 4, 2]
MTS = [DIMS[l + 1] // P for l in range(3)]  # [8, 4, 2]
# bias blob column layout: B0, B0p1, B1, B1p1, B2, B2p1
_BOFF = [0, 8, 16, 20, 24, 28]
WA_W = KTS[0] * DIMS[1] + KTS[1] * DIMS[2]  # 8192
WB_W = KTS[2] * DIMS[3] + KTS[3] * DIMS[4]  # 1152

_cache = {}


def _get_elu_op():
    """Custom DVE op: out = min(in1, max(in0 + s0, 1)).
    With in0 = z (PSUM), s0 = b+1 per-partition, in1 = exp(z+b) from ACT,
    this computes elu(z+b)+1 in a single DVE pass."""
    if "elu_op" in _cache:
        return _cache["elu_op"]
    from concourse.dve_ops import DveOp, OPS
    from concourse.dve_spec import Spec, Src0, Src1, C0, One, maxx, minn, lower
    from concourse.dve_uop import DveOpSpec

    spec = Spec(
        body=minn(Src1, maxx(Src0 + C0, One)),
        reference=lambda in0, in1, s0: np.minimum(
            in1, np.maximum(in0 + s0, 1.0)
        ),
    )
    shas = {}
    for ver in ("v3", "v4"):
        s = DveOpSpec(name="ELU_P1_MOE", opcode=0, uops=lower(spec, ver=ver), rd1_en=True)
        shas[ver] = s.sha(ver)
    op = DveOp("ELU_P1_MOE", spec, subdim=False, uops_sha=shas)
    OPS.append(op)
    # import-time lookup tables don't see post-import appends — patch them
    import concourse.dve_ops as dve_ops_mod

    dve_ops_mod.CUSTOM_DVE_SPECS[op.name] = op.spec
    dve_ops_mod._SUB_OPCODE_FOR_NAME[op.name] = (
        dve_ops_mod._CUSTOM_DVE_ROW_BASE + len(OPS) - 1
    )
    _cache["elu_op"] = op
    return op


def _build(reps=1, nb=NB, elu=None, dq=0, eb=0, dmaq=1):
    """Build the Bass graph. reps>1 wraps the whole body in a For_i loop
    (the body is idempotent) — used only for timing via wall-time slope."""
    import concourse.bass as bass  # noqa: F401
    from concourse import bacc, mybir
    import concourse.tile as tile

    _elu_override = elu

    NB = nb
    NG = NT // NB
    PS_BUFS = 8 // NB
    E_BUFS = (PS_BUFS + 2) if not eb else eb

    f32 = mybir.dt.float32
    bf16 = mybir.dt.bfloat16
    AF = mybir.ActivationFunctionType
    Alu = mybir.AluOpType

    nc = bacc.Bacc(None, target_bir_lowering=False)

    xTd = nc.dram_tensor("xT", [OBS, BSH], bf16, kind="ExternalInput")
    WAd = nc.dram_tensor("WA", [E, P, WA_W], bf16, kind="ExternalInput")
    WBd = nc.dram_tensor("WB", [E, P, WB_W], bf16, kind="ExternalInput")
    BIASd = nc.dram_tensor("BIAS", [E, P, 32], f32, kind="ExternalInput")
    gW0d = nc.dram_tensor("gW0", [OBS, GH], bf16, kind="ExternalInput")
    gW1d = nc.dram_tensor("gW1", [GH, E], bf16, kind="ExternalInput")
    # packed small constants: gb0 | gb0+1 | b3 pairs | gb1
    GSMd = nc.dram_tensor("GSM", [P, 9], f32, kind="ExternalInput")
    # out = pairs 0-2 accumulator, out2 = last pair's weighted term
    # (host sums all four 64-partition halves)
    outd = nc.dram_tensor("out", [P, BSH], f32, kind="ExternalOutput")
    out2d = nc.dram_tensor("out2", [P, BSH], f32, kind="ExternalOutput")

    with tile.TileContext(nc) as tc:
        with (
            tc.tile_pool(name="const", bufs=1) as cpool,
            tc.tile_pool(name="wapool", bufs=2) as wapool,
            tc.tile_pool(name="wbpool", bufs=4) as wbpool,
            tc.tile_pool(name="bpool", bufs=4) as bpool,
            tc.tile_pool(name="rwpool", bufs=2) as rwpool,
            tc.tile_pool(name="hpool", bufs=1) as hpool,
            tc.tile_pool(name="h3pool", bufs=2) as h3pool,
            tc.tile_pool(name="epool", bufs=E_BUFS) as epool,
            tc.tile_pool(name="rpool", bufs=4) as rpool,
            tc.tile_pool(name="tpool", bufs=2) as tpool,
            tc.tile_pool(name="psum", bufs=PS_BUFS, space="PSUM") as pspool,
            tc.tile_pool(name="dram", bufs=1, space="DRAM") as dpool,
        ):

            def body():
                # ---- load x and gate params ----
                # xt slices split across BOTH HW rings (scalar: k=0,1,3;
                # sync: k=2 ahead of the WA blob) so the gate's k-outer burn
                # rate matches slice arrivals and expert 0's weights land
                # right as the gate finishes — zero-gap handoff.
                gw0 = cpool.tile([P, OBS // P, GH], bf16, tag="gw0", name="gw0")
                nc.sync.dma_start(gw0[:], gW0d[:].rearrange("(ko p) o -> p ko o", p=P))
                xt = cpool.tile([P, OBS // P, BSH], bf16, tag="xt", name="xt")
                xt_src = xTd[:].rearrange("(ko p) n -> p ko n", p=P)
                for ko in range(OBS // P):
                    q = nc.sync if ko == 2 else nc.scalar
                    q.dma_start(xt[:, ko : ko + 1, :], xt_src[:, ko : ko + 1, :])
                gsm = cpool.tile([P, 9], f32, tag="gsm", name="gsm")
                nc.scalar.dma_start(gsm[:], GSMd[:])
                gw1 = cpool.tile([P, GH // P, E], bf16, tag="gw1", name="gw1")
                gb0t = gsm[:, 0:2]
                gb0p1t = gsm[:, 2:4]
                b3pt = gsm[:, 4:8]
                gb1t = gsm[0:E, 8:9]

                import os

                elu_mode = _elu_override or os.environ.get("ELU_MODE", "cdve")
                aux = nc.gpsimd if dmaq else nc.scalar
                elu_op = _get_elu_op() if elu_mode == "cdve" else None

                def elu_wide(ps_flat, bias_ap, biasp1_ap, out_ap, mp=P):
                    # ps_flat: [mp, NB*FD] PSUM view.
                    if elu_mode == "cdve":
                        # one wide ACT + one custom DVE from PSUM:
                        # h' = min(exp(z+b), max(z+(b+1), 1)) = elu(z+b)+1
                        et = epool.tile([P, NB * FD], bf16, tag="e", name="e")[:mp]
                        nc.scalar.activation(et, ps_flat, AF.Exp, bias=bias_ap)
                        nc.vector._custom_dve(
                            elu_op, out=out_ap, in0=ps_flat, in1=et, s0=biasp1_ap
                        )
                        return
                    # elu(z+b)+1 = min(exp(z+b), 1) + relu(z+b): two ACT passes
                    # over PSUM (released early) + one all-bf16 DVE merge
                    et = epool.tile([P, NB * FD], bf16, tag="e", name="e")[:mp]
                    nc.scalar.activation(et, ps_flat, AF.Exp, bias=bias_ap)
                    rt = rpool.tile([P, NB * FD], bf16, tag="r", name="r")[:mp]
                    nc.scalar.activation(rt, ps_flat, AF.Relu, bias=bias_ap)
                    nc.vector.scalar_tensor_tensor(
                        out_ap, et, 1.0, rt, Alu.min, Alu.add
                    )

                def psum_mm_groups(win_col, rhs_tile, KT, mp=P):
                    """All NG groups of one m-tile accumulated together so each
                    weight load serves NT consecutive matmuls (same-weight run
                    length 4). Returns one flat [mp, NB*FD] view per group."""
                    psts = [
                        pspool.tile([P, NB, FD], f32, tag="ps", name="ps")
                        for _ in range(NG)
                    ]
                    for k in range(KT):
                        lhs = win_col(k)
                        for g in range(NG):
                            for n in range(NB):
                                ng = g * NB + n
                                nc.tensor.matmul(
                                    psts[g][:mp, n, :],
                                    lhs,
                                    rhs_tile[:, k, ng * FD : (ng + 1) * FD],
                                    start=(k == 0),
                                    stop=(k == KT - 1),
                                )
                    return [pst[:mp].rearrange("p a b -> p (a b)") for pst in psts]

                def layer(win, bt, btp1, KT, MT, rhs_tile, out_tile):
                    """z = win.T @ rhs + b; out = elu(z)+1 (bf16)."""
                    for m in range(MT):
                        flats = psum_mm_groups(
                            lambda k, m=m: win[:, k, m * P : (m + 1) * P],
                            rhs_tile,
                            KT,
                        )
                        for g in range(NG):
                            elu_wide(
                                flats[g],
                                bt[:, m : m + 1],
                                btp1[:, m : m + 1],
                                out_tile[:, m, g * NB * FD : (g + 1) * NB * FD],
                            )

                def emit_gate_l1():
                    # gate layer 1 (512 -> 256, elu'), k-outer across BOTH
                    # m-tiles (4 PSUM groups): each xt k-slice is consumed in
                    # one 8-MM burst so the matmuls keep pace with the
                    # k-sliced xt DMA arrivals at kernel start.
                    gp = cpool.tile([P, GH // P, BSH], bf16, tag="gp", name="gp")
                    MT = GH // P
                    KT = OBS // P
                    psts = [
                        pspool.tile([P, NB, FD], f32, tag="ps", name="ps")
                        for _ in range(MT * NG)
                    ]
                    for k in range(KT):
                        for m in range(MT):
                            lhs = gw0[:, k, m * P : (m + 1) * P]
                            for g in range(NG):
                                for n in range(NB):
                                    ng = g * NB + n
                                    nc.tensor.matmul(
                                        psts[m * NG + g][:, n, :],
                                        lhs,
                                        xt[:, k, ng * FD : (ng + 1) * FD],
                                        start=(k == 0),
                                        stop=(k == KT - 1),
                                    )
                    for m in range(MT):
                        for g in range(NG):
                            elu_wide(
                                psts[m * NG + g].rearrange("p a b -> p (a b)"),
                                gb0t[:, m : m + 1],
                                gb0p1t[:, m : m + 1],
                                gp[:, m, g * NB * FD : (g + 1) * NB * FD],
                            )
                    return gp

                def emit_gate_rest(gp):
                    # gate layer 2 (256 -> 8) + exp
                    expT = cpool.tile([E, BSH], f32, tag="expT", name="expT")
                    gflats = psum_mm_groups(lambda k: gw1[:, k, :], gp, GH // P, mp=E)
                    for g in range(NG):
                        nc.scalar.activation(
                            expT[:, g * NB * FD : (g + 1) * NB * FD],
                            gflats[g],
                            AF.Exp,
                            bias=gb1t[:, 0:1],
                        )
                    # softmax denom: sum over 8 experts via ones-matmul
                    ones = cpool.tile([E, 1], f32, tag="ones", name="ones")
                    nc.vector.memset(ones[:], 1.0)
                    invs = cpool.tile([1, BSH], f32, tag="invs", name="invs")
                    sflats = psum_mm_groups(lambda k: ones[:], expT[:, None, :], 1, mp=1)
                    for g in range(NG):
                        nc.vector.reciprocal(
                            invs[:, g * NB * FD : (g + 1) * NB * FD], sflats[g]
                        )
                    # wT[e, s] = exp(logit_e)/sum (partition-broadcast DMA
                    # needs a DRAM source, so bounce via DRAM)
                    inv_d = dpool.tile([1, BSH], f32, name="inv_d")
                    aux.dma_start(inv_d[:], invs[:])
                    rep8 = cpool.tile([E, BSH], f32, tag="rep8", name="rep8")
                    aux.dma_start(
                        rep8[:], inv_d[0:1, :].to_broadcast((E, BSH))
                    )
                    wT = cpool.tile([E, BSH], bf16, tag="wT", name="wT")
                    nc.vector.tensor_tensor(wT[:], expT[:], rep8[:], Alu.mult)
                    wt_d = dpool.tile([E, BSH], bf16, name="wt_d")
                    aux.dma_start(wt_d[:], wT[:])
                    return wt_d

                def load_expert(e):
                    st = {}
                    wa = wapool.tile([P, WA_W], bf16, tag="wa", name="wa")
                    # W0 half first as its own transfer: L0 only needs W0,
                    # so expert 0's first layer starts ~3us sooner at kernel
                    # start / rep boundary (W1 follows before L1 needs it).
                    w0w = KTS[0] * DIMS[1]
                    nc.sync.dma_start(wa[:, 0:w0w], WAd[e][:, 0:w0w])
                    q2 = nc.scalar if dq else nc.sync
                    q2.dma_start(wa[:, w0w:WA_W], WAd[e][:, w0w:WA_W])
                    wb = wbpool.tile([P, WB_W], bf16, tag="wb", name="wb")
                    nc.sync.dma_start(wb[:], WBd[e])
                    bt = bpool.tile([P, 32], f32, tag="bias", name="bias")
                    aux.dma_start(bt[:], BIASd[e])
                    off = KTS[0] * DIMS[1]
                    st["w0"] = wa[:, 0:off].rearrange("p (k o) -> p k o", k=KTS[0])
                    st["w1"] = wa[:, off:WA_W].rearrange("p (k o) -> p k o", k=KTS[1])
                    off = KTS[2] * DIMS[3]
                    st["w2"] = wb[:, 0:off].rearrange("p (k o) -> p k o", k=KTS[2])
                    st["w3"] = wb[:, off:WB_W].rearrange("p (k o) -> p k o", k=KTS[3])
                    st["bts"] = [
                        (
                            bt[:, _BOFF[2 * l] : _BOFF[2 * l] + MTS[l]],
                            bt[:, _BOFF[2 * l + 1] : _BOFF[2 * l + 1] + MTS[l]],
                        )
                        for l in range(3)
                    ]
                    return st

                def emit_L0(st):
                    st["h1"] = hpool.tile(
                        [P, DIMS[1] // P, BSH], bf16, tag="h1", name="h1"
                    )
                    layer(
                        st["w0"], st["bts"][0][0], st["bts"][0][1],
                        KTS[0], DIMS[1] // P, xt, st["h1"],
                    )

                def emit_L1(st):
                    st["h2"] = hpool.tile(
                        [P, DIMS[2] // P, BSH], bf16, tag="h2", name="h2"
                    )
                    layer(
                        st["w1"], st["bts"][1][0], st["bts"][1][1],
                        KTS[1], DIMS[2] // P, st["h1"], st["h2"],
                    )

                def emit_L2(st):
                    st["h3"] = h3pool.tile(
                        [P, DIMS[3] // P, BSH], bf16, tag="h3", name="h3"
                    )
                    layer(
                        st["w2"], st["bts"][2][0], st["bts"][2][1],
                        KTS[2], DIMS[3] // P, st["h2"], st["h3"],
                    )

                def emit_rw_pair(pair, wt_d):
                    """Gate rows for experts (2p, 2p+1) replicated over the
                    two 64-partition halves; prefetched a tick early."""
                    eA, eB = 2 * pair, 2 * pair + 1
                    rw = rwpool.tile([P, BSH], bf16, tag="rw", name="rw")
                    aux.dma_start(
                        rw[0:ACT, :], wt_d[eA : eA + 1, :].to_broadcast((ACT, BSH))
                    )
                    aux.dma_start(
                        rw[ACT:P, :], wt_d[eB : eB + 1, :].to_broadcast((ACT, BSH))
                    )
                    return rw

                def emit_L3_pair(stA, stB, pair, acc, rw):
                    """L3 (256 -> 64) for experts (2p, 2p+1), column-tiled:
                    expert A -> PSUM partitions 0-63, B -> 64-127 so the two
                    matmul streams execute concurrently on the PE array.
                    acc[0:64] accumulates even experts, acc[64:128] odd."""
                    psts = [
                        pspool.tile([P, NB, FD], f32, tag="ps", name="ps")
                        for _ in range(NG)
                    ]
                    KT = KTS[3]
                    # Per bank: A's full k-accumulation first, then B's with
                    # its own start=True — the bank-wide has_written clear
                    # doesn't erase A's finished data and nothing rewrites
                    # A's partitions after. Adjacent banks interleave A/B so
                    # the two col-groups still overlap on the array.
                    for g in range(NG):
                        for n in range(NB):
                            ng = g * NB + n
                            rs = slice(ng * FD, (ng + 1) * FD)
                            for st_, base in ((stA, 0), (stB, ACT)):
                                for k in range(KT):
                                    nc.tensor.matmul(
                                        psts[g][base : base + ACT, n, :],
                                        st_["w3"][:, k, :],
                                        st_["h3"][:, k, rs],
                                        start=(k == 0),
                                        stop=(k == KT - 1),
                                        skip_group_check=True,
                                    )
                    for g in range(NG):
                        gs = slice(g * NB * FD, (g + 1) * NB * FD)
                        ps_flat = psts[g].rearrange("p a b -> p (a b)")
                        if pair == 0:
                            nc.vector.scalar_tensor_tensor(
                                acc[:, gs], ps_flat, b3pt[:, pair : pair + 1],
                                rw[:, gs], Alu.add, Alu.mult,
                            )
                        else:
                            tt = tpool.tile([P, NB * FD], f32, tag="t", name="t")
                            nc.vector.scalar_tensor_tensor(
                                tt, ps_flat, b3pt[:, pair : pair + 1],
                                rw[:, gs], Alu.add, Alu.mult,
                            )
                            if pair == E // 2 - 1:
                                # last pair: skip the accumulate — its term
                                # streams out on its own tensor (host sums),
                                # shortening the end-of-kernel DVE drain.
                                aux.dma_start(out2d[:, gs], tt[:])
                            else:
                                nc.vector.tensor_add(acc[:, gs], acc[:, gs], tt)

                acc = cpool.tile([P, BSH], f32, tag="acc", name="acc")

                # software pipeline over ticks t=0..E; L3 per expert pair at
                # even t covers the (t-2, t-1) experts whose h3 are both ready.
                state = {0: load_expert(0)}
                # gw1 is small and first needed at gate_rest — load it after
                # expert 0's blobs so they don't wait behind it on the ring.
                nc.sync.dma_start(gw1[:], gW1d[:].rearrange("(ko p) o -> p ko o", p=P))
                wt_d = None
                rw = None
                for t in range(E + 1):
                    if t == 0:
                        gp = emit_gate_l1()
                    if t < E:
                        if t + 1 < E:
                            state[t + 1] = load_expert(t + 1)
                        emit_L0(state[t])
                    if t == 0:
                        wt_d = emit_gate_rest(gp)
                    if t % 2 == 1:
                        # prefetch next pair's gate rows a tick early
                        rw = emit_rw_pair((t - 1) // 2, wt_d)
                    if t >= 1:
                        emit_L2(state[t - 1])
                    if t < E:
                        emit_L1(state[t])
                    if t == E:
                        # acc is final after pair 2 (tick 6) — stream it out
                        # under the last tick's compute.
                        aux.dma_start(outd[:], acc[:])
                    if t >= 2 and t % 2 == 0:
                        emit_L3_pair(
                            state[t - 2], state[t - 1], (t - 2) // 2, acc, rw
                        )
                        del state[t - 2], state[t - 1]

            if reps == 1:
                body()
            else:
                with tc.For_i(0, reps, 1):
                    body()

    nc.compile()
    return nc


def _prep_inputs(inputs):
    """Host-side: shard/transposes/casts + bias folding. Returns in_maps."""
    x = np.asarray(inputs["x"], np.float32)
    Ws = [np.asarray(inputs[f"W{l}"], np.float32) for l in range(4)]
    bs = [np.asarray(inputs[f"b{l}"], np.float32) for l in range(4)]
    gW0 = np.asarray(inputs["gW0"], np.float32)
    gb0 = np.asarray(inputs["gb0"], np.float32)
    gW1 = np.asarray(inputs["gW1"], np.float32)
    gb1 = np.asarray(inputs["gb1"], np.float32)

    shared = {}

    def kmajor(w, l):
        # [in, out] -> [P, KT, out] -> [P, KT*out]
        return (
            w.reshape(KTS[l], P, DIMS[l + 1]).transpose(1, 0, 2).reshape(P, -1)
        )

    wa = np.concatenate(
        [
            np.stack([kmajor(Ws[0][e], 0) for e in range(E)]),
            np.stack([kmajor(Ws[1][e], 1) for e in range(E)]),
        ],
        axis=2,
    )
    shared["WA"] = np.ascontiguousarray(wa.astype(BF))
    wb = np.concatenate(
        [
            np.stack([kmajor(Ws[2][e], 2) for e in range(E)]),
            np.stack([kmajor(Ws[3][e], 3) for e in range(E)]),
        ],
        axis=2,
    )
    shared["WB"] = np.ascontiguousarray(wb.astype(BF))

    # effective biases: layer l>0 consumes h' = elu+1, so subtract colsum(W_l)
    beff = [bs[0]] + [bs[l] - Ws[l].sum(axis=1) for l in range(1, 4)]
    bias = np.zeros((E, P, 32), np.float32)
    for l in range(3):
        pk = beff[l].reshape(E, MTS[l], P).transpose(0, 2, 1)
        bias[:, :, _BOFF[2 * l] : _BOFF[2 * l] + MTS[l]] = pk
        bias[:, :, _BOFF[2 * l + 1] : _BOFF[2 * l + 1] + MTS[l]] = pk + 1.0
    shared["BIAS"] = np.ascontiguousarray(bias)
    shared["gW0"] = np.ascontiguousarray(gW0.astype(BF))
    shared["gW1"] = np.ascontiguousarray(gW1.astype(BF))
    # packed small constants: gb0 | gb0+1 | b3 pairs | gb1
    gsm = np.zeros((P, 9), np.float32)
    gpk = gb0.reshape(GH // P, P).T
    gsm[:, 0:2] = gpk
    gsm[:, 2:4] = gpk + 1.0
    for p_ in range(E // 2):
        gsm[0:ACT, 4 + p_] = beff[3][2 * p_]
        gsm[ACT:P, 4 + p_] = beff[3][2 * p_ + 1]
    gsm[0:E, 8] = gb1 - gW1.sum(axis=0)
    shared["GSM"] = np.ascontiguousarray(gsm)

    in_maps = []
    for c in range(NCORES):
        m = dict(shared)
        m["xT"] = np.ascontiguousarray(x[c * BSH : (c + 1) * BSH].T.astype(BF))
        in_maps.append(m)
    return in_maps


def kernel(**inputs):
    from concourse.bass_utils import run_bass_kernel_spmd

    if "nc" not in _cache:
        _cache["nc"] = _build()
    nc = _cache["nc"]
    in_maps = _prep_inputs(inputs)
    res = run_bass_kernel_spmd(nc, in_maps, core_ids=list(range(NCORES)))
    full = np.empty((B, ACT), np.float32)
    for c in range(NCORES):
        o = np.asarray(res.results[c]["out"])
        o2 = np.asarray(res.results[c]["out2"])
        full[c * BSH : (c + 1) * BSH] = (
            o[0:ACT] + o[ACT:P] + o2[0:ACT] + o2[ACT:P]
        ).T
    return full

